# revision 14
# baseline (speedup 1.0000x reference)
"""Trainium2 Bass kernel for nn_AttentionTE_15221364097676.

Reference computation (fp32):
    xn  = LayerNorm(x) * ln_w + ln_b
    qkv = xn @ w_qkv.T -> per-head q,k,v (H=16 heads, C=64), q *= C**-0.5
    a   = softmax(q k^T + bias, masked over keys)
    y   = (a @ v).reshape(B,N,D)
    out = (sigmoid(xn @ w_g.T + b_g) * y) @ w_o.T + b_o

Sharding (8 cores): data-parallel over B (cores 0-3 -> b=0, 4-7 -> b=1),
tensor-parallel over heads (4 heads/core).  o_proj is row-parallel; the
4 partial outputs per batch are summed on the host during unsharding
(+ b_o, also host-applied).

Per-core device kernel (all layouts feature-on-partitions / d-major):
  LN stats via PE ones-matmuls on xT, normalize on DVE,
  qk^T / v / gate projections on PE (fp32r),
  scores s^T[k,q] = k^T.T @ q^T on PE, bias (bf16, host-transposed) added on
  DVE straight from PSUM, exp on ACT -> p (bf16),
  y^T = (v'|mask)^T.T @ p accumulated on PE (the extra column gives the
  softmax denominator; the key mask is folded into v'),
  normalize+gate on DVE (1/den via ACT ln/exp + PE broadcast),
  o_proj row-slice on PE.

ln_w is folded into the projection weights on the host (exact).  ln_b's
contribution enters through tiny rank-1 augmentation matmuls (qkb/vb rows);
b_g absorbs w_g @ ln_b; b_o is added on the host.
"""

import sys

for _p in ("/opt/trn_rl_repo",):
    if _p not in sys.path:
        sys.path.insert(0, _p)

from contextlib import ExitStack

import ml_dtypes
import numpy as np

import concourse.bass as bass
import concourse.tile as tile
from concourse import bacc, mybir
from concourse.bass import ds, ts

F32 = mybir.dt.float32
F32R = mybir.dt.float32r
BF16 = mybir.dt.bfloat16
AF = mybir.ActivationFunctionType
OP = mybir.AluOpType

B, N, D, H, C = 2, 2048, 1024, 16, 64
HPC = 4          # heads per core
NCORES = 8
DT = D // 128    # 8 d-tiles
NT = N // 128    # 16 token tiles
KT = N // 128    # 16 key tiles
EPS = 1e-5


def r32(ap):
    return ap.bitcast(F32R)


def _emit(tc, ctx, io):
    nc = tc.nc
    xT, wqk, wv, wg, wo, bg, mk, qkb, vb, biasT, out_p = (
        io["xT"], io["wqk"], io["wv"], io["wg"], io["wo"], io["bg"],
        io["maskk"], io["qkb"], io["vb"], io["biasT"], io["out_p"],
    )

    # ---- long-lived pools --------------------------------------------------
    const = ctx.enter_context(tc.tile_pool(name="const", bufs=1))
    qk_pool = ctx.enter_context(tc.tile_pool(name="qkT", bufs=1))
    v_pool = ctx.enter_context(tc.tile_pool(name="v2", bufs=1))
    g_pool = ctx.enter_context(tc.tile_pool(name="gate", bufs=1))

    # ---- constants ---------------------------------------------------------
    wo_sb = const.tile([128, 2, 1024], F32R)
    nc.sync.dma_start(wo_sb[:], wo.rearrange("(t p) e -> p t e", p=128))
    ones_f = const.tile([128, 128], F32)
    nc.vector.memset(ones_f[:], 1.0)
    ones_sb = const.tile([128, 128], F32R)
    nc.vector.tensor_copy(ones_sb[:], ones_f[:])

    with tc.tile_pool(name="xt", bufs=1) as xpool, \
         tc.tile_pool(name="wts", bufs=1) as wts, \
         tc.tile_pool(name="stats", bufs=1) as stats, \
         tc.tile_pool(name="sq", bufs=2) as sqpool, \
         tc.tile_pool(name="lnrow", bufs=2) as lnrow, \
         tc.tile_pool(name="lnps", bufs=2, space="PSUM") as lnps, \
         tc.tile_pool(name="qkps", bufs=2, space="PSUM") as qkps:

        wqk_sb = wts.tile([128, DT, 512], F32R)
        nc.sync.dma_start(wqk_sb[:], wqk.rearrange("(dt p) m -> p dt m", p=128))
        wv_sb = wts.tile([128, DT, 256], F32R)
        nc.sync.dma_start(wv_sb[:], wv.rearrange("(dt p) m -> p dt m", p=128))
        wg_sb = wts.tile([128, DT, 256], F32R)
        nc.sync.dma_start(wg_sb[:], wg.rearrange("(dt p) m -> p dt m", p=128))
        bg_sb = wts.tile([128, 2], F32)
        nc.sync.dma_start(bg_sb[:], bg)
        mk_sb = wts.tile([128, KT], F32)
        nc.sync.dma_start(mk_sb[:], mk)
        qkb_sb = wts.tile([1, 512], F32R)
        nc.sync.dma_start(qkb_sb[:], qkb)
        vb_sb = wts.tile([1, 256], F32R)
        nc.sync.dma_start(vb_sb[:], vb)
        ones_row_f = wts.tile([1, 512], F32)
        nc.vector.memset(ones_row_f[:], 1.0)
        ones_row = wts.tile([1, 512], F32R)
        nc.vector.tensor_copy(ones_row[:], ones_row_f[:])
        eps_sb = wts.tile([128, 1], F32)
        nc.vector.memset(eps_sb[:], EPS)
        xt = xpool.tile([128, DT, N], F32R)
        nc.sync.dma_start(xt[:], xT.rearrange("(dt p) n -> p dt n", p=128))

        # ---- Phase 1: LayerNorm stats + normalize (d-major) ---------------
        # sum rows via PE ones-matmul, broadcast back via ones outer-product
        mu_b = stats.tile([128, N], F32)
        var_b = stats.tile([128, N], F32)
        rstd_b = var_b
        for c4 in range(4):
            sp = lnps.tile([1, 512], F32, tag="lnrowps")
            for dt in range(DT):
                nc.tensor.matmul(sp[:], r32(ones_sb[:, 0:1]),
                                 r32(xt[:, dt, ts(c4, 512)]),
                                 start=(dt == 0), stop=(dt == DT - 1))
            rowt = lnrow.tile([1, 512], F32R, tag="rowt")
            nc.scalar.copy(rowt[:], sp[:])
            bp = lnps.tile([128, 512], F32, tag="lnbps")
            nc.tensor.matmul(bp[:], r32(ones_sb[0:1, :]), r32(rowt[:]),
                             start=True, stop=True)
            nc.vector.tensor_scalar(out=mu_b[:, ts(c4, 512)], in0=bp[:],
                                    scalar1=1.0 / D, scalar2=None, op0=OP.mult)
        for c4 in range(4):
            sp = lnps.tile([1, 512], F32, tag="lnrowps")
            for dt in range(DT):
                sq = sqpool.tile([128, 512], F32R)
                nc.scalar.activation(sq[:], xt[:, dt, ts(c4, 512)], AF.Square)
                nc.tensor.matmul(sp[:], r32(ones_sb[:, 0:1]), r32(sq[:]),
                                 start=(dt == 0), stop=(dt == DT - 1))
            rowt = lnrow.tile([1, 512], F32R, tag="rowt")
            nc.scalar.copy(rowt[:], sp[:])
            bp2 = lnps.tile([128, 512], F32, tag="lnbps")
            nc.tensor.matmul(bp2[:], r32(ones_sb[0:1, :]), r32(rowt[:]),
                             start=True, stop=True)
            # var = s2/D - mu^2  (E[x^2] - mu^2)
            mu2 = sqpool.tile([128, 512], F32, tag="mu2")
            nc.vector.tensor_mul(mu2[:], mu_b[:, ts(c4, 512)], mu_b[:, ts(c4, 512)])
            nc.vector.scalar_tensor_tensor(out=var_b[:, ts(c4, 512)], in0=bp2[:],
                                           scalar=1.0 / D, in1=mu2[:],
                                           op0=OP.mult, op1=OP.subtract)
        # rstd = exp(-0.5 * ln(var + eps))
        nc.scalar.activation(rstd_b[:], var_b[:], AF.Ln, bias=eps_sb[:], scale=1.0)
        nc.scalar.activation(rstd_b[:], rstd_b[:], AF.Exp, scale=-0.5)
        # xn = (x - mu) * rstd, in place
        for dt in range(DT):
            nc.vector.tensor_sub(xt[:, dt, :], xt[:, dt, :], mu_b[:])
            nc.vector.tensor_mul(xt[:, dt, :], xt[:, dt, :], rstd_b[:])

        # ---- Phase 2: q/k projections -> qkT [e, n] ------------------------
        # Mtile order: [qP0(A|B), kP0(A|B), qP1(A|B), kP1(A|B)]
        qkT = qk_pool.tile([128, 4, N], F32R)
        for mt in range(4):
            for c4 in range(4):
                ps = qkps.tile([128, 512], F32)
                for dt in range(DT):
                    nc.tensor.matmul(ps[:], r32(wqk_sb[:, dt, ts(mt, 128)]),
                                     r32(xt[:, dt, ts(c4, 512)]),
                                     start=(dt == 0), stop=False)
                nc.tensor.matmul(ps[:], r32(qkb_sb[:, ts(mt, 128)]),
                                 r32(ones_row[:]), start=False, stop=True)
                nc.scalar.copy(qkT[:, mt, ts(c4, 512)], ps[:])

        # ---- Phase 3: v projection -> v2 [k, pair, (vA|m | m|vB)] ---------
        # layout per (kt, pair): [vA*m(64) | m(1) | vB*m(64)] = 129 cols
        v2 = v_pool.tile([128, KT, 2, 129], BF16)
        mkb = v_pool.tile([128, KT], BF16)
        nc.vector.tensor_copy(mkb[:], mk_sb[:])
        for p in range(2):
            nc.vector.tensor_copy(
                v2[:, :, p, 64:65].rearrange("p k o -> p (k o)"), mk_sb[:])
        for nt in range(NT):
            ps = qkps.tile([128, 256], F32, tag="vps")
            for dt in range(DT):
                nc.tensor.matmul(ps[:], r32(xt[:, dt, ts(nt, 128)]),
                                 r32(wv_sb[:, dt, :]),
                                 start=(dt == 0), stop=False)
            nc.tensor.matmul(ps[:], r32(ones_sb[0:1, :]),
                             r32(vb_sb[:]), start=False, stop=True)
            for p in range(2):
                nc.vector.tensor_scalar(
                    out=v2[:, nt, p, 0:64], in0=ps[:, ds(p * 128, 64)],
                    scalar1=mk_sb[:, nt:nt + 1], scalar2=None, op0=OP.mult)
                nc.vector.tensor_scalar(
                    out=v2[:, nt, p, 65:129], in0=ps[:, ds(p * 128 + 64, 64)],
                    scalar1=mk_sb[:, nt:nt + 1], scalar2=None, op0=OP.mult)

        # ---- Phase 4: gate = sigmoid(wg @ xn + bg) -> g [gcol, n] ----------
        g_sb = g_pool.tile([128, 2, N], F32)
        for gt in range(2):
            for c4 in range(4):
                ps = qkps.tile([128, 512], F32)
                for dt in range(DT):
                    nc.tensor.matmul(ps[:], r32(wg_sb[:, dt, ts(gt, 128)]),
                                     r32(xt[:, dt, ts(c4, 512)]),
                                     start=(dt == 0), stop=(dt == DT - 1))
                nc.scalar.activation(g_sb[:, gt, ts(c4, 512)], ps[:], AF.Sigmoid,
                                     bias=bg_sb[:, gt:gt + 1], scale=1.0)

    # ---- Phase 5: attention -----------------------------------------------
    yg_pool = ctx.enter_context(tc.tile_pool(name="yg", bufs=1))
    yg = yg_pool.tile([128, 2, N], F32R)
    att = ExitStack()
    bias_pool = att.enter_context(tc.tile_pool(name="bias", bufs=6))
    sps_pool = att.enter_context(tc.tile_pool(name="sps", bufs=2, space="PSUM"))
    yps_pool = att.enter_context(tc.tile_pool(name="yps", bufs=2, space="PSUM"))
    ssb_pool = att.enter_context(tc.tile_pool(name="ssb", bufs=4))
    p_pool = att.enter_context(tc.tile_pool(name="pexp", bufs=4))
    row_pool = att.enter_context(tc.tile_pool(name="rows", bufs=2))
    for pair in range(2):
        qmt, kmt = 2 * pair, 2 * pair + 1
        for qc in range(2):
            # bias tiles, interleaved A/B per kt-group of 4
            bts = {}
            for ktg in range(4):
                for h in range(2):
                    bt = bias_pool.tile([128, 4, 1024], BF16)
                    hi = 2 * pair + h
                    nc.sync.dma_start(
                        bt[:],
                        biasT[hi, ds(ktg * 512, 512), ds(qc * 1024, 1024)]
                        .rearrange("(g p) q -> p g q", p=128))
                    bts[(ktg, h)] = bt
            yp = [yps_pool.tile([128, 1024], F32, tag="yp", name="yp") for _ in range(2)]
            for kt in range(KT):
                ktg, gidx = kt // 4, kt % 4
                s_ps = [sps_pool.tile([128, 1024], F32, tag="sps", name="sps") for _ in range(2)]
                for h in range(2):
                    base = h * 64
                    for half in range(2):
                        nc.tensor.matmul(
                            s_ps[h][:, ts(half, 512)],
                            r32(qkT[base:base + 64, kmt, ts(kt, 128)]),
                            r32(qkT[base:base + 64, qmt,
                                    ds(qc * 1024 + half * 512, 512)]),
                            start=True, stop=True)
                s_sb = [ssb_pool.tile([128, 1024], F32, tag="ssb", name="ssb") for _ in range(2)]
                for h in range(2):
                    nc.vector.tensor_tensor(out=s_sb[h][:], in0=s_ps[h][:],
                                            in1=bts[(ktg, h)][:, gidx, :],
                                            op=OP.add)
                p_t = [p_pool.tile([128, 1024], BF16, tag="pt", name="pt") for _ in range(2)]
                for h in range(2):
                    nc.scalar.activation(p_t[h][:], s_sb[h][:], AF.Exp)
                # head A: [vA*m | m] -> rows 0:65 (den at 64)
                for half in range(2):
                    nc.tensor.matmul(yp[0][0:65, ts(half, 512)],
                                     v2[:, kt, pair, 0:65],
                                     p_t[0][:, ts(half, 512)],
                                     start=(kt == 0), stop=(kt == KT - 1))
                # head B: y rows 64:128; den via M=1 mask matmul into rows 0:1
                for half in range(2):
                    nc.tensor.matmul(yp[1][64:128, ts(half, 512)],
                                     v2[:, kt, pair, 65:129],
                                     p_t[1][:, ts(half, 512)],
                                     start=(kt == 0), stop=(kt == KT - 1))
                    nc.tensor.matmul(yp[1][0:1, ts(half, 512)],
                                     mkb[:, kt:kt + 1],
                                     p_t[1][:, ts(half, 512)],
                                     start=(kt == 0), stop=(kt == KT - 1))
            # epilogue: yg = (y / den) * g
            for h in range(2):
                dpart = 64 if h == 0 else 0
                ylo = h * 64
                rden = row_pool.tile([128, 1024], F32R)
                nc.scalar.activation(rden[dpart:dpart + 1, :],
                                     yp[h][dpart:dpart + 1, :], AF.Ln)
                nc.scalar.activation(rden[dpart:dpart + 1, :],
                                     rden[dpart:dpart + 1, :], AF.Exp, scale=-1.0)
                rb = sps_pool.tile([128, 1024], F32, tag="sps", name="sps")
                if h == 0:
                    # den row at partition 64 -> broadcast to rows 0:64
                    for half in range(2):
                        nc.tensor.matmul(rb[0:64, ts(half, 512)],
                                         r32(ones_sb[64:65, 0:64]),
                                         r32(rden[64:65, ts(half, 512)]),
                                         start=True, stop=True)
                else:
                    # den row at partition 0 -> broadcast to all 128 rows
                    for half in range(2):
                        nc.tensor.matmul(rb[:, ts(half, 512)],
                                         r32(ones_sb[0:1, :]),
                                         r32(rden[0:1, ts(half, 512)]),
                                         start=True, stop=True)
                geff = ssb_pool.tile([128, 1024], F32, tag="ssb", name="ssb")
                nc.vector.tensor_tensor(out=geff[ylo:ylo + 64, :],
                                        in0=rb[ylo:ylo + 64, :],
                                        in1=g_sb[ylo:ylo + 64, pair,
                                                 ds(qc * 1024, 1024)],
                                        op=OP.mult)
                nc.vector.tensor_tensor(out=yg[ylo:ylo + 64, pair,
                                               ds(qc * 1024, 1024)],
                                        in0=yp[h][ylo:ylo + 64, :],
                                        in1=geff[ylo:ylo + 64, :], op=OP.mult)

    att.close()

    # ---- Phase 6: o_proj (row-parallel slice) ------------------------------
    with tc.tile_pool(name="ops", bufs=2, space="PSUM") as ops_pool, \
         tc.tile_pool(name="outsb", bufs=2) as out_pool:
        for nt in range(NT):
            ps = ops_pool.tile([128, 1024], F32)
            for half in range(2):
                for pt in range(2):
                    nc.tensor.matmul(ps[:, ts(half, 512)],
                                     r32(yg[:, pt, ts(nt, 128)]),
                                     r32(wo_sb[:, pt, ds(half * 512, 512)]),
                                     start=(pt == 0), stop=(pt == 1))
            ot = out_pool.tile([128, 1024], F32)
            nc.scalar.copy(ot[:], ps[:])
            nc.sync.dma_start(out_p[ds(nt * 128, 128), :], ot[:])


_CACHED = None


def build_program():
    global _CACHED
    if _CACHED is not None:
        return _CACHED
    nc = bacc.Bacc("TRN2", target_bir_lowering=False, debug=False,
                   enable_asserts=False, num_devices=NCORES)
    io = {
        "xT": nc.dram_tensor("xT", (D, N), F32R, kind="ExternalInput").ap(),
        "wqk": nc.dram_tensor("wqk", (D, 512), F32R, kind="ExternalInput").ap(),
        "wv": nc.dram_tensor("wv", (D, 256), F32R, kind="ExternalInput").ap(),
        "wg": nc.dram_tensor("wg", (D, 256), F32R, kind="ExternalInput").ap(),
        "wo": nc.dram_tensor("wo", (256, D), F32R, kind="ExternalInput").ap(),
        "bg": nc.dram_tensor("bg", (128, 2), F32, kind="ExternalInput").ap(),
        "maskk": nc.dram_tensor("maskk", (128, KT), F32, kind="ExternalInput").ap(),
        "qkb": nc.dram_tensor("qkb", (1, 512), F32R, kind="ExternalInput").ap(),
        "vb": nc.dram_tensor("vb", (1, 256), F32R, kind="ExternalInput").ap(),
        "biasT": nc.dram_tensor("biasT", (HPC, N, N), BF16,
                                kind="ExternalInput").ap(),
        "out_p": nc.dram_tensor("out_p", (N, D), F32, kind="ExternalOutput").ap(),
    }
    with tile.TileContext(nc) as tc, ExitStack() as ctx:
        _emit(tc, ctx, io)
    nc.compile()
    _CACHED = nc
    return nc


def prep_in_maps(x, bias, mask, ln_w, ln_b, w_qkv, w_o, b_o, w_g, b_g):
    """Host-side sharding: slice/transpose/reorder/cast only (plus exact
    folds of ln_w / ln_b / q-scale into weights, which are O(params))."""
    x = np.asarray(x, np.float32)
    bias = np.asarray(bias, np.float32)
    mask = np.asarray(mask)
    ln_w = np.asarray(ln_w, np.float32)
    ln_b = np.asarray(ln_b, np.float32)
    w_qkv = np.asarray(w_qkv, np.float32)
    w_o = np.asarray(w_o, np.float32)
    w_g = np.asarray(w_g, np.float32)
    b_g = np.asarray(b_g, np.float32)

    wql = w_qkv * ln_w[None, :]          # ln_w fold (exact)
    wgl = w_g * ln_w[None, :]
    qkv_lb = w_qkv @ ln_b                # ln_b rank-1 corrections
    g_lb = w_g @ ln_b
    qscale = C ** -0.5

    in_maps = []
    for core in range(NCORES):
        b = core // 4
        h0 = HPC * (core % 4)
        # qk weight Mtiles: [qP0, kP0, qP1, kP1], each [A(64)|B(64)] cols
        qk_rows, qk_scale = [], []
        for pair in range(2):
            hA, hB = h0 + 2 * pair, h0 + 2 * pair + 1
            for off, sc in ((0, qscale), (64, 1.0)):
                for h in (hA, hB):
                    qk_rows.extend(range(h * 192 + off, h * 192 + off + 64))
                    qk_scale.extend([sc] * 64)
        qk_rows = np.array(qk_rows)
        qk_scale = np.array(qk_scale, np.float32)
        v_rows = np.concatenate(
            [np.arange(h * 192 + 128, h * 192 + 192) for h in range(h0, h0 + 4)])
        d0 = 64 * h0

        wqk_c = np.ascontiguousarray((wql[qk_rows] * qk_scale[:, None]).T)
        wv_c = np.ascontiguousarray(wql[v_rows].T)
        wg_c = np.ascontiguousarray(wgl[d0:d0 + 256].T)
        wo_c = np.ascontiguousarray(w_o[:, d0:d0 + 256].T)
        bg_c = np.ascontiguousarray(
            (b_g + g_lb)[d0:d0 + 256].reshape(2, 128).T)
        mk_c = np.ascontiguousarray(
            mask[b].astype(np.float32).reshape(KT, 128).T)
        qkb_c = (qkv_lb[qk_rows] * qk_scale).reshape(1, 512)
        vb_c = qkv_lb[v_rows].reshape(1, 256)
        biasT_c = np.ascontiguousarray(
            bias[b, h0:h0 + 4].transpose(0, 2, 1)).astype(ml_dtypes.bfloat16)
        xT_c = np.ascontiguousarray(x[b].T)

        in_maps.append({
            "xT": xT_c, "wqk": wqk_c, "wv": wv_c, "wg": wg_c, "wo": wo_c,
            "bg": bg_c, "maskk": mk_c,
            "qkb": np.ascontiguousarray(qkb_c, dtype=np.float32),
            "vb": np.ascontiguousarray(vb_c, dtype=np.float32),
            "biasT": biasT_c,
        })
    return in_maps


def gather(results, b_o):
    b_o = np.asarray(b_o, np.float32)
    out = np.zeros((B, N, D), np.float32)
    for core, res in enumerate(results):
        out[core // 4] += res["out_p"]
    out += b_o[None, None, :]
    return out


def run(inputs, **spmd_kwargs):
    from concourse import bass_utils
    nc = build_program()
    in_maps = prep_in_maps(**inputs)
    res = bass_utils.run_bass_kernel_spmd(
        nc, in_maps, core_ids=list(range(NCORES)), **spmd_kwargs)
    return gather(res.results, inputs["b_o"]), res


def kernel(**inputs) -> np.ndarray:
    out, _ = run(inputs)
    return out
